# revision 14
# baseline (speedup 1.0000x reference)
"""Trainium2 Bass kernel for nn_Embed_38766374814290 (embedding_lookup).

Math: out[i,j,l,e] = A[m][e] + delta_s[i,j,l] * B[m][e]
  where m = (j < traj_len[i]), delta_s = where(m, mat2[traj_loc-1], 0),
  A[m] = emb_sl_w[m] + emb_tl_w[m],
  B[m] = (emb_su_w[m]-emb_sl_w[m])/SU + (emb_tu_w[m]-emb_tl_w[m])/TU.

Sharding: pure data parallel over batch N = 32 -> 4 rows per core x 8 cores.

The rel-err gate is 2e-2; bf16 output rounding is ~2^-9, so the device
computes and stores the output in bf16 (halving HBM write traffic vs
fp32 -> ~23us DMA roofline per core) and the host upcasts to fp32.

Per-core pipeline, per batch row i (128 positions):
  1. One transposing dma_gather pulls 128 rows of a padded 256-wide
     table (window g of a row = [mat2x[loc, 32g:32g+32] | m-mark | 1 |
     0-pad] at cols 64g:64g+64; invalid positions redirect to row 4096
     whose data and m-mark are 0 but 1-marks are 1). The xbar transpose
     lands it as gt[p, h, q] = table[idx[q]][128h + p]: partitions = l,
     free = positions -- lhsT layout directly, no on-chip transposes.
  2. Matmul per (g, s): lhsT = gt[64*(g&1) : +34, g>>1, :] (K=34 incl
     the m/1 rows -> A[m] added in-matmul), rhs = block-diagonal B1
     tiles (duplicated at partition base 64 for odd windows). Four
     s-matmuls (F=512) fill a [128, 2048] 4-bank PSUM tile per group.
  3. One wide [128, 2048] PSUM->SBUF eviction per (i, g) casts fp32 to
     bf16 (alternating Activation/Vector engines).
  4. Output DMA per (i, g): natural [pos, l*E] layout, 128 x 4KiB
     contiguous descriptors, spread over all 16 DMA queues.
"""
import os
import numpy as np
from contextlib import ExitStack

SU, TU = 10000.0, 86400.0
N, M, L, E = 32, 128, 128, 64
NLOC = 4096
NCORES = 8
ROWS = N // NCORES  # 4 batch rows per core

_CACHE = {}


def _install_profhook():
    """Optional: shim the missing antenv.axon_hooks so trace=True works."""
    import sys
    import types
    if "antenv.axon_hooks" in sys.modules:
        return True
    try:
        from trn_agent_boot.trn_boot import _ntff_profile_via_ctypes
    except Exception:
        return False
    hook = [None]
    mod = types.ModuleType("antenv.axon_hooks")
    mod.set_axon_ntff_profile_hook = lambda h: hook.__setitem__(0, h)
    mod.get_axon_ntff_profile_hook = lambda: hook[0]
    sys.modules["antenv.axon_hooks"] = mod
    try:
        mod.set_axon_ntff_profile_hook(
            _ntff_profile_via_ctypes("/opt/axon/libaxon_pjrt.so"))
    except Exception:
        return False
    return True


def _build():
    import concourse.bass as bass
    import concourse.tile as tile
    from concourse import bacc, mybir

    F32 = mybir.dt.float32
    BF16 = mybir.dt.bfloat16
    I16 = mybir.dt.int16

    nc = bacc.Bacc("TRN2", target_bir_lowering=False, debug=False,
                   enable_asserts=True, num_devices=NCORES)
    m2_d = nc.dram_tensor("m2", [NLOC + 1, 256], BF16,
                          kind="ExternalInput").ap()
    idx_d = nc.dram_tensor("idx", [128, 8 * ROWS], I16,
                           kind="ExternalInput").ap()
    rhs_d = nc.dram_tensor("rhs", [4, 34, 8 * E], BF16,
                           kind="ExternalInput").ap()
    out_d = nc.dram_tensor("out", [ROWS, M, L * E], BF16,
                           kind="ExternalOutput").ap()

    with tile.TileContext(nc) as tc, ExitStack() as ctx:
        const = ctx.enter_context(tc.tile_pool(name="const", bufs=1))
        gpool = ctx.enter_context(tc.tile_pool(name="gath", bufs=1))
        opool = ctx.enter_context(tc.tile_pool(name="orow", bufs=3))
        pso = ctx.enter_context(tc.tile_pool(name="pso", bufs=2, space="PSUM"))

        idxt = const.tile([128, 8 * ROWS], I16)
        nc.sync.dma_start(idxt[:], idx_d[:])

        # The first DMAGatherAnt on hardware stalls ~9us on the lazy
        # gpsimd ucode library load. A dependency-free dummy gather
        # absorbs that stall while the inputs load and the PE warms up.
        dix = const.tile([128, 8], I16, tag="dix")
        nc.vector.memset(dix[:], 0)
        dgt = gpool.tile([128, 2, 128], BF16, tag="dummy")
        nc.gpsimd.dma_gather(
            out_ap=dgt[:], in_ap=m2_d[:], idxs_ap=dix[:],
            num_idxs=128, num_idxs_reg=128, elem_size=256, transpose=True)

        # one transposing gather for all 4 rows: gtall[p, h, 128i+q] =
        # table[idx_i[q]][128h + p] -- lhsT layout directly
        gtall = gpool.tile([128, 2, 128 * ROWS], BF16, tag="gt")
        nc.gpsimd.dma_gather(
            out_ap=gtall[:], in_ap=m2_d[:], idxs_ap=idxt[:],
            num_idxs=128 * ROWS, num_idxs_reg=128 * ROWS, elem_size=256,
            transpose=True)

        # rhs tiles at partition base 0 (windows g=0,2) and 64 (g=1,3)
        rlo, rhi = [], []
        for s in range(4):
            rt = const.tile([34, 8 * E], BF16, tag=f"rlo{s}")
            nc.scalar.dma_start(rt[:], rhs_d[s])
            rlo.append(rt)
        for s in range(4):
            rt = const.tile([98, 8 * E], BF16, tag=f"rhi{s}")
            nc.scalar.dma_start(rt[64:98, :], rhs_d[s])
            rhi.append(rt)

        # HAM warmup: back-to-back matmuls lift the PE clock gate before
        # the real burst. Results are never read.
        wlhs = const.tile([128, 128], BF16)
        nc.vector.memset(wlhs[:], 0.0)
        wrhs = const.tile([128, 8 * E], BF16)
        nc.vector.memset(wrhs[:], 0.0)
        wpo = pso.tile([128, 4 * 8 * E], F32, tag="po")
        for _ in range(20):
            nc.tensor.matmul(wpo[:, 0:512], lhsT=wlhs[:], rhs=wrhs[:],
                             start=True, stop=True)

        # evict engine pattern: 9 scalar(ACT) / 7 vector(DVE)
        epat = [1, 0, 1, 0, 1, 0, 1, 1, 0, 1, 0, 1, 0, 1, 1, 0]

        for i in range(ROWS):
            for g in range(4):
                base = 64 * (g & 1)
                h = g >> 1
                po = pso.tile([128, 4 * 8 * E], F32, tag="po")
                w = slice(128 * i, 128 * (i + 1))
                for s in range(4):
                    if base == 0:
                        nc.tensor.matmul(po[:, 512 * s:512 * (s + 1)],
                                         lhsT=gtall[0:34, h, w],
                                         rhs=rlo[s][:],
                                         start=True, stop=True)
                    else:
                        nc.tensor.matmul(po[:, 512 * s:512 * (s + 1)],
                                         lhsT=gtall[64:98, h, w],
                                         rhs=rhi[s][64:98, :],
                                         start=True, stop=True)
                orow = opool.tile([128, 4 * 8 * E], BF16)
                if epat[4 * i + g]:
                    nc.scalar.copy(out=orow[:], in_=po[:])
                else:
                    nc.vector.tensor_copy(out=orow[:], in_=po[:])
                nc.sync.dma_start(out_d[i][:, 2048 * g:2048 * (g + 1)],
                                  orow[:])
    nc.compile()
    return nc


def kernel(traj_loc, mat2, vec, traj_len, l_max, emb_sl_w, emb_su_w,
           emb_tl_w, emb_tu_w):
    import ml_dtypes
    from concourse import bass_utils

    BF = ml_dtypes.bfloat16
    traj_loc = np.asarray(traj_loc).astype(np.int64)
    mat2 = np.ascontiguousarray(np.asarray(mat2, dtype=np.float32))
    traj_len = np.asarray(traj_len).astype(np.int64)
    esl = np.asarray(emb_sl_w, dtype=np.float32)
    esu = np.asarray(emb_su_w, dtype=np.float32)
    etl = np.asarray(emb_tl_w, dtype=np.float32)
    etu = np.asarray(emb_tu_w, dtype=np.float32)

    # host prep: constants
    A = esl + etl                                            # [2, E]
    B = (esu - esl) / np.float32(SU) + (etu - etl) / np.float32(TU)
    mask = (np.arange(M)[None, :] < traj_len[:, None])       # [N, M]
    idx_full = np.where(mask, traj_loc - 1, NLOC).astype(np.int32)

    b1 = B[1].astype(BF)
    dA = (A[1] - A[0]).astype(BF)
    a0 = A[0].astype(BF)

    # padded gather table: window g = [ds 32 | m-mark | 1-mark | 0 pad]
    tab = np.zeros((NLOC + 1, 256), np.float32)
    for g in range(4):
        tab[:NLOC, 64 * g:64 * g + 32] = mat2[:, 32 * g:32 * (g + 1)]
        tab[:NLOC, 64 * g + 32] = 1.0   # m-marker: 0 in the zero-row
        tab[:, 64 * g + 33] = 1.0       # 1-marker: 1 everywhere
    tabBF = np.ascontiguousarray(tab.astype(BF))

    # idx wrapped in 16 partitions: idx16[p, 8i+s] = idx[i][16s + p%16]
    idx16 = np.empty((NCORES, 128, 8 * ROWS), np.int16)
    p16 = np.arange(128) % 16
    for c in range(NCORES):
        for i in range(ROWS):
            idx = idx_full[ROWS * c + i]
            for s in range(8):
                idx16[c, :, 8 * i + s] = idx[16 * s + p16]

    # rhs[s] is [34, 512]: row 8s+lp has B1 in e-block lp; rows 32/33
    # pair with lhsT rows [m, 1]: out += m*dA + A0 in every e-block.
    rhs = np.zeros((4, 34, 8 * E), BF)
    for s in range(4):
        for lp in range(8):
            rhs[s, 8 * s + lp, E * lp:E * (lp + 1)] = b1
        rhs[s, 32, :] = np.tile(dA, 8)
        rhs[s, 33, :] = np.tile(a0, 8)

    if "nc" not in _CACHE:
        _CACHE["nc"] = _build()
    nc = _CACHE["nc"]

    in_maps = []
    for c in range(NCORES):
        in_maps.append({
            "m2": tabBF,
            "idx": np.ascontiguousarray(idx16[c]),
            "rhs": rhs,
        })

    trace = os.environ.get("KERNEL_TRACE", "0") == "1" and _install_profhook()
    res = bass_utils.run_bass_kernel_spmd(
        nc, in_maps, core_ids=list(range(NCORES)), trace=bool(trace))
    if trace:
        _CACHE["exec_time_ns"] = res.exec_time_ns
        _CACHE["trace_path"] = (res.instructions_and_trace or (None, None))[1]
        _CACHE["tmpdir"] = res.profile_json

    out = np.concatenate(
        [res.results[c]["out"].reshape(ROWS, M, L, E) for c in range(NCORES)],
        axis=0).astype(np.float32)
    return out


# revision 20
# speedup vs baseline: 1.1162x; 1.1162x over previous
"""Trainium2 Bass kernel for nn_Embed_38766374814290 (embedding_lookup).

Math: out[i,j,l,e] = A[m][e] + delta_s[i,j,l] * B[m][e]
  where m = (j < traj_len[i]), delta_s = where(m, mat2[traj_loc-1], 0),
  A[m] = emb_sl_w[m] + emb_tl_w[m],
  B[m] = (emb_su_w[m]-emb_sl_w[m])/SU + (emb_tu_w[m]-emb_tl_w[m])/TU.

Sharding: pure data parallel over batch N = 32 -> 4 rows per core x 8 cores.

The rel-err gate is 2e-2; bf16 output rounding is ~2^-9, so the device
computes and stores the output in bf16 (halving HBM write traffic vs
fp32 -> ~23us DMA roofline per core) and the host upcasts to fp32.

Per-core pipeline, per batch row i (128 positions):
  1. One transposing dma_gather pulls 128 rows of a padded 256-wide
     table (window g of a row = [mat2x[loc, 32g:32g+32] | m-mark | 1 |
     0-pad] at cols 64g:64g+64; invalid positions redirect to row 4096
     whose data and m-mark are 0 but 1-marks are 1). The xbar transpose
     lands it as gt[p, h, q] = table[idx[q]][128h + p]: partitions = l,
     free = positions -- lhsT layout directly, no on-chip transposes.
  2. Matmul per (g, s): lhsT = gt[64*(g&1) : +34, g>>1, :] (K=34 incl
     the m/1 rows -> A[m] added in-matmul), rhs = block-diagonal B1
     tiles (duplicated at partition base 64 for odd windows). Four
     s-matmuls (F=512) fill a [128, 2048] 4-bank PSUM tile per group.
  3. One wide [128, 2048] PSUM->SBUF eviction per (i, g) casts fp32 to
     bf16 (alternating Activation/Vector engines).
  4. Output DMA per (i, g): natural [pos, l*E] layout, 128 x 4KiB
     contiguous descriptors, spread over all 16 DMA queues.
"""
import os
import numpy as np
from contextlib import ExitStack

SU, TU = 10000.0, 86400.0
N, M, L, E = 32, 128, 128, 64
NLOC = 4096
NCORES = 8
ROWS = N // NCORES  # 4 batch rows per core

_CACHE = {}


def _install_profhook():
    """Optional: shim the missing antenv.axon_hooks so trace=True works."""
    import sys
    import types
    if "antenv.axon_hooks" in sys.modules:
        return True
    try:
        from trn_agent_boot.trn_boot import _ntff_profile_via_ctypes
    except Exception:
        return False
    hook = [None]
    mod = types.ModuleType("antenv.axon_hooks")
    mod.set_axon_ntff_profile_hook = lambda h: hook.__setitem__(0, h)
    mod.get_axon_ntff_profile_hook = lambda: hook[0]
    sys.modules["antenv.axon_hooks"] = mod
    try:
        mod.set_axon_ntff_profile_hook(
            _ntff_profile_via_ctypes("/opt/axon/libaxon_pjrt.so"))
    except Exception:
        return False
    return True


def _build():
    import concourse.bass as bass
    import concourse.tile as tile
    from concourse import bacc, mybir

    F32 = mybir.dt.float32
    BF16 = mybir.dt.bfloat16
    I16 = mybir.dt.int16

    nc = bacc.Bacc("TRN2", target_bir_lowering=False, debug=False,
                   enable_asserts=True, num_devices=NCORES)
    m2_d = nc.dram_tensor("m2", [NLOC + 1, 256], BF16,
                          kind="ExternalInput").ap()
    idx_d = nc.dram_tensor("idx", [128, 8 * ROWS], I16,
                           kind="ExternalInput").ap()
    rhs_d = nc.dram_tensor("rhs", [4, 34, 8 * E], BF16,
                           kind="ExternalInput").ap()
    out_d = nc.dram_tensor("out", [ROWS, M, L * E], BF16,
                           kind="ExternalOutput").ap()

    with tile.TileContext(nc) as tc, ExitStack() as ctx:
        const = ctx.enter_context(tc.tile_pool(name="const", bufs=1))
        gpool = ctx.enter_context(tc.tile_pool(name="gath", bufs=4))
        opool = ctx.enter_context(tc.tile_pool(name="orow", bufs=3))
        pso = ctx.enter_context(tc.tile_pool(name="pso", bufs=4, space="PSUM"))

        idxt = const.tile([128, 8 * ROWS], I16)
        nc.sync.dma_start(idxt[:], idx_d[:])

        # per-row transposing gathers: gts[i][p, h, q] = table[idx_i[q]]
        # [128h + p] -- lhsT layout directly. The first one stalls ~11us
        # on the lazy gpsimd ucode library load (hardware behavior);
        # row 0 lands first so the matmul pipeline starts ASAP.
        gts = []
        for i in range(ROWS):
            gt = gpool.tile([128, 2, 128], BF16)
            nc.gpsimd.dma_gather(
                out_ap=gt[:], in_ap=m2_d[:],
                idxs_ap=idxt[:, 8 * i:8 * (i + 1)],
                num_idxs=128, num_idxs_reg=128, elem_size=256,
                transpose=True)
            gts.append(gt)

        # rhs tiles at partition base 0 (windows g=0,2) and 64 (g=1,3)
        rlo, rhi = [], []
        for s in range(4):
            rt = const.tile([34, 8 * E], BF16, tag=f"rlo{s}")
            nc.scalar.dma_start(rt[:], rhs_d[s])
            rlo.append(rt)
        for s in range(4):
            rt = const.tile([98, 8 * E], BF16, tag=f"rhi{s}")
            nc.scalar.dma_start(rt[64:98, :], rhs_d[s])
            rhi.append(rt)

        # HAM warmup: back-to-back matmuls lift the PE clock gate before
        # the real burst. lhsT = the (bitcast) idx tile, so the warmup
        # starts only once the idx DMA lands (~9.5us) and bridges the
        # PE-busy window to the first real matmul (~20us) without a
        # re-throttling gap. Results are never read.
        wrhs = const.tile([128, 8 * E], BF16)
        nc.vector.memset(wrhs[:], 0.0)
        wpo = pso.tile([128, 2 * 8 * E], F32, tag="po")
        wlhs = idxt[:].bitcast(BF16)
        for _ in range(40):
            nc.tensor.matmul(wpo[0:32, 0:512], lhsT=wlhs[:, 0:32],
                             rhs=wrhs[:], start=True, stop=True)

        # per group: two 2-bank PSUM tiles (s-pairs), evicted in parallel
        # on Activation and Vector so the per-group eviction latency is
        # ~1.2us and four tiles of WAR depth keep the PE fed
        for i in range(ROWS):
            for g in range(4):
                base = 64 * (g & 1)
                h = g >> 1
                orow = opool.tile([128, 4 * 8 * E], BF16)
                for half in range(2):
                    po = pso.tile([128, 2 * 8 * E], F32, tag="po")
                    for sp in range(2):
                        s = 2 * half + sp
                        if base == 0:
                            nc.tensor.matmul(po[:, 512 * sp:512 * (sp + 1)],
                                             lhsT=gts[i][0:34, h, :],
                                             rhs=rlo[s][:],
                                             start=True, stop=True)
                        else:
                            nc.tensor.matmul(po[:, 512 * sp:512 * (sp + 1)],
                                             lhsT=gts[i][64:98, h, :],
                                             rhs=rhi[s][64:98, :],
                                             start=True, stop=True)
                    dst = orow[:, 1024 * half:1024 * (half + 1)]
                    if half == 0:
                        nc.scalar.copy(out=dst, in_=po[:])
                    else:
                        nc.vector.tensor_copy(out=dst, in_=po[:])
                nc.sync.dma_start(out_d[i][:, 2048 * g:2048 * (g + 1)],
                                  orow[:])
    nc.compile()
    return nc


def kernel(traj_loc, mat2, vec, traj_len, l_max, emb_sl_w, emb_su_w,
           emb_tl_w, emb_tu_w):
    import ml_dtypes
    from concourse import bass_utils

    BF = ml_dtypes.bfloat16
    traj_loc = np.asarray(traj_loc).astype(np.int64)
    mat2 = np.ascontiguousarray(np.asarray(mat2, dtype=np.float32))
    traj_len = np.asarray(traj_len).astype(np.int64)
    esl = np.asarray(emb_sl_w, dtype=np.float32)
    esu = np.asarray(emb_su_w, dtype=np.float32)
    etl = np.asarray(emb_tl_w, dtype=np.float32)
    etu = np.asarray(emb_tu_w, dtype=np.float32)

    # host prep: constants
    A = esl + etl                                            # [2, E]
    B = (esu - esl) / np.float32(SU) + (etu - etl) / np.float32(TU)
    mask = (np.arange(M)[None, :] < traj_len[:, None])       # [N, M]
    idx_full = np.where(mask, traj_loc - 1, NLOC).astype(np.int32)

    b1 = B[1].astype(BF)
    dA = (A[1] - A[0]).astype(BF)
    a0 = A[0].astype(BF)

    # padded gather table: window g = [ds 32 | m-mark | 1-mark | 0 pad]
    tab = np.zeros((NLOC + 1, 256), np.float32)
    for g in range(4):
        tab[:NLOC, 64 * g:64 * g + 32] = mat2[:, 32 * g:32 * (g + 1)]
        tab[:NLOC, 64 * g + 32] = 1.0   # m-marker: 0 in the zero-row
        tab[:, 64 * g + 33] = 1.0       # 1-marker: 1 everywhere
    tabBF = np.ascontiguousarray(tab.astype(BF))

    # idx wrapped in 16 partitions: idx16[p, 8i+s] = idx[i][16s + p%16]
    idx16 = np.empty((NCORES, 128, 8 * ROWS), np.int16)
    p16 = np.arange(128) % 16
    for c in range(NCORES):
        for i in range(ROWS):
            idx = idx_full[ROWS * c + i]
            for s in range(8):
                idx16[c, :, 8 * i + s] = idx[16 * s + p16]

    # rhs[s] is [34, 512]: row 8s+lp has B1 in e-block lp; rows 32/33
    # pair with lhsT rows [m, 1]: out += m*dA + A0 in every e-block.
    rhs = np.zeros((4, 34, 8 * E), BF)
    for s in range(4):
        for lp in range(8):
            rhs[s, 8 * s + lp, E * lp:E * (lp + 1)] = b1
        rhs[s, 32, :] = np.tile(dA, 8)
        rhs[s, 33, :] = np.tile(a0, 8)

    if "nc" not in _CACHE:
        _CACHE["nc"] = _build()
    nc = _CACHE["nc"]

    in_maps = []
    for c in range(NCORES):
        in_maps.append({
            "m2": tabBF,
            "idx": np.ascontiguousarray(idx16[c]),
            "rhs": rhs,
        })

    trace = os.environ.get("KERNEL_TRACE", "0") == "1" and _install_profhook()
    res = bass_utils.run_bass_kernel_spmd(
        nc, in_maps, core_ids=list(range(NCORES)), trace=bool(trace))
    if trace:
        _CACHE["exec_time_ns"] = res.exec_time_ns
        _CACHE["trace_path"] = (res.instructions_and_trace or (None, None))[1]
        _CACHE["tmpdir"] = res.profile_json

    out = np.concatenate(
        [res.results[c]["out"].reshape(ROWS, M, L, E) for c in range(NCORES)],
        axis=0).astype(np.float32)
    return out


# revision 21
# speedup vs baseline: 1.1662x; 1.0448x over previous
"""Trainium2 Bass kernel for nn_Embed_38766374814290 (embedding_lookup).

Math: out[i,j,l,e] = A[m][e] + delta_s[i,j,l] * B[m][e]
  where m = (j < traj_len[i]), delta_s = where(m, mat2[traj_loc-1], 0),
  A[m] = emb_sl_w[m] + emb_tl_w[m],
  B[m] = (emb_su_w[m]-emb_sl_w[m])/SU + (emb_tu_w[m]-emb_tl_w[m])/TU.

Sharding: pure data parallel over batch N = 32 -> 4 rows per core x 8 cores.

The rel-err gate is 2e-2; bf16 output rounding is ~2^-9, so the device
computes and stores the output in bf16 (halving HBM write traffic vs
fp32 -> ~23us DMA roofline per core) and the host upcasts to fp32.

Per-core pipeline, per batch row i, per 32-position block j:
  1. One indirect row-gather pulls mat2x rows for the 32 positions of
     block j into gsw[32, 128j:+128] (invalid positions redirect to an
     appended all-zero row 4096). No DMAGatherAnt: the gpsimd SWDGE
     descriptor path needs no ucode library, so gathers start ~9.5us.
  2. A DVE stream-transpose (in-place 32x32 blocks) turns the j-window
     into lhsT layout: gt[a, w+32g+b] = ds[pos=32j+b, l=32g+a]. Rows
     32:34 of gtrow ([m, 1] per column) come from one host-prepared
     DMA; they add A[m] inside the matmul.
  3. Matmul (i,j,s): lhsT = the contiguous window [34, 128]; out
     partition f = 32g+b carries (pos=32j+b, l-group g). Two s-matmuls
     (K=34, F=512) per 2-bank PSUM tile, two tiles per group, so the
     Activation and Vector engines evict the halves in parallel
     (~1.15us latency) and 4-tile WAR depth keeps the PE fed.
  4. Output DMA per (i, j) writes the permuted [128, 2048] tile as-is
     (128 x 4KiB contiguous descriptors over all 16 DMA queues); the
     host gather step undoes the (g,b) permutation while upcasting.
"""
import os
import numpy as np
from contextlib import ExitStack

SU, TU = 10000.0, 86400.0
N, M, L, E = 32, 128, 128, 64
NLOC = 4096
NCORES = 8
ROWS = N // NCORES  # 4 batch rows per core

_CACHE = {}


def _install_profhook():
    """Optional: shim the missing antenv.axon_hooks so trace=True works."""
    import sys
    import types
    if "antenv.axon_hooks" in sys.modules:
        return True
    try:
        from trn_agent_boot.trn_boot import _ntff_profile_via_ctypes
    except Exception:
        return False
    hook = [None]
    mod = types.ModuleType("antenv.axon_hooks")
    mod.set_axon_ntff_profile_hook = lambda h: hook.__setitem__(0, h)
    mod.get_axon_ntff_profile_hook = lambda: hook[0]
    sys.modules["antenv.axon_hooks"] = mod
    try:
        mod.set_axon_ntff_profile_hook(
            _ntff_profile_via_ctypes("/opt/axon/libaxon_pjrt.so"))
    except Exception:
        return False
    return True


def _build():
    import concourse.bass as bass
    import concourse.tile as tile
    from concourse import bacc, mybir

    F32 = mybir.dt.float32
    BF16 = mybir.dt.bfloat16
    I32 = mybir.dt.int32

    nc = bacc.Bacc("TRN2", target_bir_lowering=False, debug=False,
                   enable_asserts=True, num_devices=NCORES)
    m2_d = nc.dram_tensor("m2", [NLOC + 1, L], BF16,
                          kind="ExternalInput").ap()
    offs_d = nc.dram_tensor("offs", [32, 4 * ROWS], I32,
                            kind="ExternalInput").ap()
    mrow_d = nc.dram_tensor("mrow", [2, ROWS * 512], BF16,
                            kind="ExternalInput").ap()
    rhs_d = nc.dram_tensor("rhs", [4, 34, 8 * E], BF16,
                           kind="ExternalInput").ap()
    # device-side layout keeps the matmul partition permutation:
    # out[i, j, 32g+b, 512s+64lp+e] = result(pos=32j+b, l=32g+8s+lp, e)
    out_d = nc.dram_tensor("out", [ROWS, 4, M, 4 * 8 * E], BF16,
                           kind="ExternalOutput").ap()

    with tile.TileContext(nc) as tc, ExitStack() as ctx:
        const = ctx.enter_context(tc.tile_pool(name="const", bufs=1))
        gpool = ctx.enter_context(tc.tile_pool(name="gath", bufs=2))
        opool = ctx.enter_context(tc.tile_pool(name="orow", bufs=3))
        pso = ctx.enter_context(tc.tile_pool(name="pso", bufs=4, space="PSUM"))

        offt = const.tile([32, 4 * ROWS], I32)
        nc.sync.dma_start(offt[:], offs_d[:])
        # gtrow holds all lhsT windows [34, ROWS*4*128]; rows 0:32 are
        # G^T blocks (written by stream transposes), rows 32:34 = [m, 1]
        gtrow = const.tile([34, ROWS * 512], BF16)
        nc.scalar.dma_start(gtrow[32:34, :], mrow_d[:])
        rhs_tiles = []
        for s in range(4):
            rt = const.tile([34, 8 * E], BF16, tag=f"rhs{s}")
            nc.scalar.dma_start(rt[:], rhs_d[s])
            rhs_tiles.append(rt)

        # HAM warmup: a few matmuls gated on the offs DMA (~9.5us) keep
        # the PE clock ramping until the first real matmul arrives.
        # Results are never read.
        wrhs = const.tile([32, 8 * E], BF16)
        nc.vector.memset(wrhs[:], 0.0)
        wpo = pso.tile([128, 2 * 8 * E], F32, tag="po")
        wlhs = offt[:].bitcast(BF16)
        for _ in range(6):
            nc.tensor.matmul(wpo[0:32, 0:512], lhsT=wlhs[:, 0:32],
                             rhs=wrhs[:], start=True, stop=True)

        # evict pattern per group: (ACT, DVE) halves in parallel; two
        # groups go (ACT, ACT) to offload the stream-transpose-loaded DVE
        act_both = {5, 11}

        for i in range(ROWS):
            gsw = gpool.tile([32, 512], BF16)
            for j in range(4):
                nc.gpsimd.indirect_dma_start(
                    out=gsw[:, 128 * j:128 * (j + 1)], out_offset=None,
                    in_=m2_d[:],
                    in_offset=bass.IndirectOffsetOnAxis(
                        ap=offt[:, 4 * i + j:4 * i + j + 1], axis=0))
                if i == 0:
                    # per-j transpose lets row 0's first group start
                    # right after its first gather
                    nc.vector.transpose(
                        out=gtrow[0:32, 128 * j:128 * (j + 1)],
                        in_=gsw[:, 128 * j:128 * (j + 1)])
            if i > 0:
                nc.vector.transpose(out=gtrow[0:32, 512 * i:512 * (i + 1)],
                                    in_=gsw[:])
            for j in range(4):
                w = 512 * i + 128 * j
                k = 4 * i + j
                orow = opool.tile([128, 4 * 8 * E], BF16)
                for half in range(2):
                    po = pso.tile([128, 2 * 8 * E], F32, tag="po")
                    for sp in range(2):
                        s = 2 * half + sp
                        nc.tensor.matmul(po[:, 512 * sp:512 * (sp + 1)],
                                         lhsT=gtrow[:, w:w + 128],
                                         rhs=rhs_tiles[s][:],
                                         start=True, stop=True)
                    dst = orow[:, 1024 * half:1024 * (half + 1)]
                    if half == 0 or k in act_both:
                        nc.scalar.copy(out=dst, in_=po[:])
                    else:
                        nc.vector.tensor_copy(out=dst, in_=po[:])
                nc.sync.dma_start(out_d[i, j], orow[:])
    nc.compile()
    return nc


def kernel(traj_loc, mat2, vec, traj_len, l_max, emb_sl_w, emb_su_w,
           emb_tl_w, emb_tu_w):
    import ml_dtypes
    from concourse import bass_utils

    BF = ml_dtypes.bfloat16
    traj_loc = np.asarray(traj_loc).astype(np.int64)
    mat2 = np.ascontiguousarray(np.asarray(mat2, dtype=np.float32))
    traj_len = np.asarray(traj_len).astype(np.int64)
    esl = np.asarray(emb_sl_w, dtype=np.float32)
    esu = np.asarray(emb_su_w, dtype=np.float32)
    etl = np.asarray(emb_tl_w, dtype=np.float32)
    etu = np.asarray(emb_tu_w, dtype=np.float32)

    # host prep: constants
    A = esl + etl                                            # [2, E]
    B = (esu - esl) / np.float32(SU) + (etu - etl) / np.float32(TU)
    mask = (np.arange(M)[None, :] < traj_len[:, None])       # [N, M]
    idx_full = np.where(mask, traj_loc - 1, NLOC).astype(np.int32)

    b1 = B[1].astype(BF)
    dA = (A[1] - A[0]).astype(BF)
    a0 = A[0].astype(BF)

    mat2x = np.concatenate([mat2, np.zeros((1, L), np.float32)], axis=0)
    m2 = np.ascontiguousarray(mat2x.astype(BF))

    # gather offsets: gather (i, j) row-gathers mat2x[idx[i, 32j+p]] into
    # partition p of gsw[:, 128j:128j+128]
    offs = np.empty((NCORES, 32, 4 * ROWS), np.int32)
    for c in range(NCORES):
        for i in range(ROWS):
            idx = idx_full[ROWS * c + i]                     # [128]
            for j in range(4):
                offs[c, :, 4 * i + j] = idx[32 * j:32 * (j + 1)]

    # rhs[s] is [34, 512]: row 8s+lp has B1 in e-block lp (selects the
    # lp-th l within each partition's own l-group); rows 32/33 pair with
    # lhsT rows [m, 1]: out += m*dA + A0, tiled across all 8 e-blocks.
    rhs = np.zeros((4, 34, 8 * E), BF)
    for s in range(4):
        for lp in range(8):
            rhs[s, 8 * s + lp, E * lp:E * (lp + 1)] = b1
        rhs[s, 32, :] = np.tile(dA, 8)
        rhs[s, 33, :] = np.tile(a0, 8)

    # gtrow rows 32:34: col 512i+128j+32g+b must hold m[pos=32j+b] -> the
    # j-th 32-chunk of mask, tiled 4x (over g), per (i, j)
    mrow_full = np.empty((NCORES, 2, ROWS * 512), BF)
    for c in range(NCORES):
        mc = mask[ROWS * c:ROWS * (c + 1)].astype(BF)        # [ROWS, 128]
        mrow_full[c, 0, :] = np.tile(mc.reshape(ROWS, 4, 1, 32),
                                     (1, 1, 4, 1)).reshape(-1)
        mrow_full[c, 1, :] = 1.0

    if "nc" not in _CACHE:
        _CACHE["nc"] = _build()
    nc = _CACHE["nc"]

    in_maps = []
    for c in range(NCORES):
        in_maps.append({
            "m2": m2,
            "offs": np.ascontiguousarray(offs[c]),
            "mrow": np.ascontiguousarray(mrow_full[c]),
            "rhs": rhs,
        })

    trace = os.environ.get("KERNEL_TRACE", "0") == "1" and _install_profhook()
    res = bass_utils.run_bass_kernel_spmd(
        nc, in_maps, core_ids=list(range(NCORES)), trace=bool(trace))
    if trace:
        _CACHE["exec_time_ns"] = res.exec_time_ns
        _CACHE["trace_path"] = (res.instructions_and_trace or (None, None))[1]
        _CACHE["tmpdir"] = res.profile_json

    # undo the device layout: [i, j, g, b, s, lp, e] -> [i, (j b), (g s lp), e]
    out = np.concatenate(
        [res.results[c]["out"].reshape(ROWS, 4, 4, 32, 4, 8, E)
         .transpose(0, 1, 3, 2, 4, 5, 6).reshape(ROWS, M, L, E)
         for c in range(NCORES)],
        axis=0).astype(np.float32)
    return out


# revision 24
# speedup vs baseline: 1.3184x; 1.1305x over previous
"""Trainium2 Bass kernel for nn_Embed_38766374814290 (embedding_lookup).

Math: out[i,j,l,e] = A[m][e] + delta_s[i,j,l] * B[m][e]
  where m = (j < traj_len[i]), delta_s = where(m, mat2[traj_loc-1], 0),
  A[m] = emb_sl_w[m] + emb_tl_w[m],
  B[m] = (emb_su_w[m]-emb_sl_w[m])/SU + (emb_tu_w[m]-emb_tl_w[m])/TU.

Sharding: pure data parallel over batch N = 32 -> 4 rows per core x 8 cores.

The rel-err gate is 2e-2; bf16 output rounding is ~2^-9, so the device
computes and stores the output in bf16 (halving HBM write traffic vs
fp32 -> ~23us DMA roofline per core) and the host upcasts to fp32.

Per-core pipeline, per batch row i, per 32-position block j:
  1. One indirect row-gather pulls mat2x rows for the 32 positions of
     block j into gsw[32, 128j:+128] (invalid positions redirect to an
     appended all-zero row 4096). No DMAGatherAnt: the gpsimd SWDGE
     descriptor path needs no ucode library, so gathers start ~9.5us.
  2. A DVE stream-transpose (in-place 32x32 blocks) turns the j-window
     into lhsT layout: gt[a, w+32g+b] = ds[pos=32j+b, l=32g+a]. Rows
     32:34 of gtrow ([m, 1] per column) come from one host-prepared
     DMA; they add A[m] inside the matmul.
  3. Matmul (i,j,s): lhsT = the contiguous window [34, 128]; out
     partition f = 32g+b carries (pos=32j+b, l-group g). Two s-matmuls
     (K=34, F=512) per 2-bank PSUM tile, two tiles per group, so the
     Activation and Vector engines evict the halves in parallel
     (~1.15us latency) and 4-tile WAR depth keeps the PE fed.
  4. Output DMA per (i, j) writes the permuted [128, 2048] tile as-is
     (128 x 4KiB contiguous descriptors over all 16 DMA queues); the
     host gather step undoes the (g,b) permutation while upcasting.
"""
import os
import numpy as np
from contextlib import ExitStack

SU, TU = 10000.0, 86400.0
N, M, L, E = 32, 128, 128, 64
NLOC = 4096
NCORES = 8
ROWS = N // NCORES  # 4 batch rows per core

_CACHE = {}


def _install_profhook():
    """Optional: shim the missing antenv.axon_hooks so trace=True works."""
    import sys
    import types
    if "antenv.axon_hooks" in sys.modules:
        return True
    try:
        from trn_agent_boot.trn_boot import _ntff_profile_via_ctypes
    except Exception:
        return False
    hook = [None]
    mod = types.ModuleType("antenv.axon_hooks")
    mod.set_axon_ntff_profile_hook = lambda h: hook.__setitem__(0, h)
    mod.get_axon_ntff_profile_hook = lambda: hook[0]
    sys.modules["antenv.axon_hooks"] = mod
    try:
        mod.set_axon_ntff_profile_hook(
            _ntff_profile_via_ctypes("/opt/axon/libaxon_pjrt.so"))
    except Exception:
        return False
    return True


def _build():
    import concourse.bass as bass
    import concourse.tile as tile
    from concourse import bacc, mybir

    F32 = mybir.dt.float32
    BF16 = mybir.dt.bfloat16
    I32 = mybir.dt.int32

    nc = bacc.Bacc("TRN2", target_bir_lowering=False, debug=False,
                   enable_asserts=True, num_devices=NCORES)
    m2_d = nc.dram_tensor("m2", [NLOC + 1, L], BF16,
                          kind="ExternalInput").ap()
    offs_d = nc.dram_tensor("offs", [32, 4 * ROWS], I32,
                            kind="ExternalInput").ap()
    mrow_d = nc.dram_tensor("mrow", [2, ROWS * 512], BF16,
                            kind="ExternalInput").ap()
    rhs_d = nc.dram_tensor("rhs", [4, 34, 8 * E], BF16,
                           kind="ExternalInput").ap()
    # device-side layout keeps the matmul partition permutation:
    # out[i, j, 32g+b, 512s+64lp+e] = result(pos=32j+b, l=32g+8s+lp, e)
    out_d = nc.dram_tensor("out", [ROWS, 4, M, 4 * 8 * E], BF16,
                           kind="ExternalOutput").ap()

    with tile.TileContext(nc) as tc, ExitStack() as ctx:
        const = ctx.enter_context(tc.tile_pool(name="const", bufs=1))
        gpool = ctx.enter_context(tc.tile_pool(name="gath", bufs=2))
        opool = ctx.enter_context(tc.tile_pool(name="orow", bufs=6))
        pso = ctx.enter_context(tc.tile_pool(name="pso", bufs=4, space="PSUM"))

        offt = const.tile([32, 4 * ROWS], I32)
        nc.sync.dma_start(offt[:], offs_d[:])
        # gtrow holds all lhsT windows [34, ROWS*4*128]; rows 0:32 are
        # G^T blocks (written by stream transposes), rows 32:34 = [m, 1]
        gtrow = const.tile([34, ROWS * 512], BF16)
        nc.scalar.dma_start(gtrow[32:34, :], mrow_d[:])
        rhs_tiles = []
        for s in range(4):
            rt = const.tile([34, 8 * E], BF16, tag=f"rhs{s}")
            nc.scalar.dma_start(rt[:], rhs_d[s])
            rhs_tiles.append(rt)

        # HAM warmup: a few matmuls gated on the offs DMA (~9.5us) keep
        # the PE clock ramping until the first real matmul arrives.
        # Results are never read.
        wrhs = const.tile([32, 8 * E], BF16)
        nc.vector.memset(wrhs[:], 0.0)
        wpo = pso.tile([128, 2 * 8 * E], F32, tag="po")
        wlhs = offt[:].bitcast(BF16)
        for _ in range(9):
            nc.tensor.matmul(wpo[0:32, 0:512], lhsT=wlhs[:, 0:32],
                             rhs=wrhs[:], start=True, stop=True)

        # evict pattern per group: (ACT, DVE) halves in parallel; two
        # groups go (ACT, ACT) to offload the stream-transpose-loaded DVE
        act_both = {5, 11}

        for i in range(ROWS):
            gsw = gpool.tile([32, 512], BF16)
            for j in range(4):
                # per-j transpose lets each group start right after its
                # own gather; on the in-order DVE queue these interleave
                # with the earlier rows' eviction casts without stalling
                nc.gpsimd.indirect_dma_start(
                    out=gsw[:, 128 * j:128 * (j + 1)], out_offset=None,
                    in_=m2_d[:],
                    in_offset=bass.IndirectOffsetOnAxis(
                        ap=offt[:, 4 * i + j:4 * i + j + 1], axis=0))
                nc.vector.transpose(
                    out=gtrow[0:32, 512 * i + 128 * j:512 * i + 128 * (j + 1)],
                    in_=gsw[:, 128 * j:128 * (j + 1)])
            for j in range(4):
                w = 512 * i + 128 * j
                k = 4 * i + j
                orow = opool.tile([128, 4 * 8 * E], BF16)
                for half in range(2):
                    po = pso.tile([128, 2 * 8 * E], F32, tag="po")
                    for sp in range(2):
                        s = 2 * half + sp
                        nc.tensor.matmul(po[:, 512 * sp:512 * (sp + 1)],
                                         lhsT=gtrow[:, w:w + 128],
                                         rhs=rhs_tiles[s][:],
                                         start=True, stop=True)
                    dst = orow[:, 1024 * half:1024 * (half + 1)]
                    if half == 0 or k in act_both:
                        nc.scalar.copy(out=dst, in_=po[:])
                    else:
                        nc.vector.tensor_copy(out=dst, in_=po[:])
                nc.sync.dma_start(out_d[i, j], orow[:])
    nc.compile()
    return nc


def kernel(traj_loc, mat2, vec, traj_len, l_max, emb_sl_w, emb_su_w,
           emb_tl_w, emb_tu_w):
    import ml_dtypes
    from concourse import bass_utils

    BF = ml_dtypes.bfloat16
    traj_loc = np.asarray(traj_loc).astype(np.int64)
    mat2 = np.ascontiguousarray(np.asarray(mat2, dtype=np.float32))
    traj_len = np.asarray(traj_len).astype(np.int64)
    esl = np.asarray(emb_sl_w, dtype=np.float32)
    esu = np.asarray(emb_su_w, dtype=np.float32)
    etl = np.asarray(emb_tl_w, dtype=np.float32)
    etu = np.asarray(emb_tu_w, dtype=np.float32)

    # host prep: constants
    A = esl + etl                                            # [2, E]
    B = (esu - esl) / np.float32(SU) + (etu - etl) / np.float32(TU)
    mask = (np.arange(M)[None, :] < traj_len[:, None])       # [N, M]
    idx_full = np.where(mask, traj_loc - 1, NLOC).astype(np.int32)

    b1 = B[1].astype(BF)
    dA = (A[1] - A[0]).astype(BF)
    a0 = A[0].astype(BF)

    mat2x = np.concatenate([mat2, np.zeros((1, L), np.float32)], axis=0)
    m2 = np.ascontiguousarray(mat2x.astype(BF))

    # gather offsets: gather (i, j) row-gathers mat2x[idx[i, 32j+p]] into
    # partition p of gsw[:, 128j:128j+128]
    offs = np.empty((NCORES, 32, 4 * ROWS), np.int32)
    for c in range(NCORES):
        for i in range(ROWS):
            idx = idx_full[ROWS * c + i]                     # [128]
            for j in range(4):
                offs[c, :, 4 * i + j] = idx[32 * j:32 * (j + 1)]

    # rhs[s] is [34, 512]: row 8s+lp has B1 in e-block lp (selects the
    # lp-th l within each partition's own l-group); rows 32/33 pair with
    # lhsT rows [m, 1]: out += m*dA + A0, tiled across all 8 e-blocks.
    rhs = np.zeros((4, 34, 8 * E), BF)
    for s in range(4):
        for lp in range(8):
            rhs[s, 8 * s + lp, E * lp:E * (lp + 1)] = b1
        rhs[s, 32, :] = np.tile(dA, 8)
        rhs[s, 33, :] = np.tile(a0, 8)

    # gtrow rows 32:34: col 512i+128j+32g+b must hold m[pos=32j+b] -> the
    # j-th 32-chunk of mask, tiled 4x (over g), per (i, j)
    mrow_full = np.empty((NCORES, 2, ROWS * 512), BF)
    for c in range(NCORES):
        mc = mask[ROWS * c:ROWS * (c + 1)].astype(BF)        # [ROWS, 128]
        mrow_full[c, 0, :] = np.tile(mc.reshape(ROWS, 4, 1, 32),
                                     (1, 1, 4, 1)).reshape(-1)
        mrow_full[c, 1, :] = 1.0

    if "nc" not in _CACHE:
        _CACHE["nc"] = _build()
    nc = _CACHE["nc"]

    in_maps = []
    for c in range(NCORES):
        in_maps.append({
            "m2": m2,
            "offs": np.ascontiguousarray(offs[c]),
            "mrow": np.ascontiguousarray(mrow_full[c]),
            "rhs": rhs,
        })

    trace = os.environ.get("KERNEL_TRACE", "0") == "1" and _install_profhook()
    res = bass_utils.run_bass_kernel_spmd(
        nc, in_maps, core_ids=list(range(NCORES)), trace=bool(trace))
    if trace:
        _CACHE["exec_time_ns"] = res.exec_time_ns
        _CACHE["trace_path"] = (res.instructions_and_trace or (None, None))[1]
        _CACHE["tmpdir"] = res.profile_json

    # undo the device layout: [i, j, g, b, s, lp, e] -> [i, (j b), (g s lp), e]
    out = np.concatenate(
        [res.results[c]["out"].reshape(ROWS, 4, 4, 32, 4, 8, E)
         .transpose(0, 1, 3, 2, 4, 5, 6).reshape(ROWS, M, L, E)
         for c in range(NCORES)],
        axis=0).astype(np.float32)
    return out


# revision 27
# speedup vs baseline: 1.3991x; 1.0612x over previous
"""Trainium2 Bass kernel for nn_Embed_38766374814290 (embedding_lookup).

Math: out[i,j,l,e] = A[m][e] + delta_s[i,j,l] * B[m][e]
  where m = (j < traj_len[i]), delta_s = where(m, mat2[traj_loc-1], 0),
  A[m] = emb_sl_w[m] + emb_tl_w[m],
  B[m] = (emb_su_w[m]-emb_sl_w[m])/SU + (emb_tu_w[m]-emb_tl_w[m])/TU.

Sharding: pure data parallel over batch N = 32 -> 4 rows per core x 8 cores.

The rel-err gate is 2e-2; bf16 output rounding is ~2^-9, so the device
computes and stores the output in bf16 (halving HBM write traffic vs
fp32 -> ~23us DMA roofline per core) and the host upcasts to fp32.

Per-core pipeline, per batch row i, per 32-position block j:
  1. One indirect row-gather pulls mat2x rows for the 32 positions of
     block j into gsw[32, 128j:+128] (invalid positions redirect to an
     appended all-zero row 4096). No DMAGatherAnt: the gpsimd SWDGE
     descriptor path needs no ucode library, so gathers start ~9.5us.
  2. A DVE stream-transpose (in-place 32x32 blocks) turns the j-window
     into lhsT layout: gt[a, w+32g+b] = ds[pos=32j+b, l=32g+a]. Rows
     32:34 of gtrow ([m, 1] per column) come from one host-prepared
     DMA; they add A[m] inside the matmul.
  3. Matmul (i,j,s): lhsT = the contiguous window [34, 128]; out
     partition f = 32g+b carries (pos=32j+b, l-group g). Two s-matmuls
     (K=34, F=512) per 2-bank PSUM tile, two tiles per group, so the
     Activation and Vector engines evict the halves in parallel
     (~1.15us latency) and 4-tile WAR depth keeps the PE fed.
  4. Output DMA per (i, j) writes the permuted [128, 2048] tile as-is
     (128 x 4KiB contiguous descriptors over all 16 DMA queues); the
     host gather step undoes the (g,b) permutation while upcasting.
"""
import os
import numpy as np
from contextlib import ExitStack

SU, TU = 10000.0, 86400.0
N, M, L, E = 32, 128, 128, 64
NLOC = 4096
NCORES = 8
ROWS = N // NCORES  # 4 batch rows per core

_CACHE = {}


def _install_profhook():
    """Optional: shim the missing antenv.axon_hooks so trace=True works."""
    import sys
    import types
    if "antenv.axon_hooks" in sys.modules:
        return True
    try:
        from trn_agent_boot.trn_boot import _ntff_profile_via_ctypes
    except Exception:
        return False
    hook = [None]
    mod = types.ModuleType("antenv.axon_hooks")
    mod.set_axon_ntff_profile_hook = lambda h: hook.__setitem__(0, h)
    mod.get_axon_ntff_profile_hook = lambda: hook[0]
    sys.modules["antenv.axon_hooks"] = mod
    try:
        mod.set_axon_ntff_profile_hook(
            _ntff_profile_via_ctypes("/opt/axon/libaxon_pjrt.so"))
    except Exception:
        return False
    return True


def _build():
    import concourse.bass as bass
    import concourse.tile as tile
    from concourse import bacc, mybir

    F32 = mybir.dt.float32
    BF16 = mybir.dt.bfloat16
    I32 = mybir.dt.int32

    nc = bacc.Bacc("TRN2", target_bir_lowering=False, debug=False,
                   enable_asserts=True, num_devices=NCORES)
    m2_d = nc.dram_tensor("m2", [NLOC + 1, L], BF16,
                          kind="ExternalInput").ap()
    offs_d = nc.dram_tensor("offs", [32, 4 * ROWS], I32,
                            kind="ExternalInput").ap()
    mrow_d = nc.dram_tensor("mrow", [96, ROWS * 512], BF16,
                            kind="ExternalInput").ap()
    rhs_d = nc.dram_tensor("rhs", [4, 128, 8 * E], BF16,
                           kind="ExternalInput").ap()
    # device-side layout keeps the matmul partition permutation:
    # out[i, j, 32g+b, 512s+64lp+e] = result(pos=32j+b, l=32g+8s+lp, e)
    out_d = nc.dram_tensor("out", [ROWS, 4, M, 4 * 8 * E], BF16,
                           kind="ExternalOutput").ap()

    with tile.TileContext(nc) as tc, ExitStack() as ctx:
        const = ctx.enter_context(tc.tile_pool(name="const", bufs=1))
        gpool = ctx.enter_context(tc.tile_pool(name="gath", bufs=2))
        opool = ctx.enter_context(tc.tile_pool(name="orow", bufs=6))
        pso = ctx.enter_context(tc.tile_pool(name="pso", bufs=4, space="PSUM"))

        offt = const.tile([32, 4 * ROWS], I32)
        nc.sync.dma_start(offt[:], offs_d[:])
        # gtrow holds all lhsT windows [128, ROWS*4*128]; rows 0:32 are
        # G^T blocks (written by stream transposes), rows 32:34 = [m, 1],
        # rows 34:128 zero padding: K=128 matmuls use the full PE array
        # (K<=64 tiles run at half column throughput)
        gtrow = const.tile([128, ROWS * 512], BF16)
        nc.scalar.dma_start(gtrow[32:128, :], mrow_d[:])
        rhs_tiles = []
        for s in range(4):
            rt = const.tile([128, 8 * E], BF16, tag=f"rhs{s}")
            nc.scalar.dma_start(rt[:], rhs_d[s])
            rhs_tiles.append(rt)

        # HAM warmup: a few matmuls gated on the offs DMA (~9.5us) keep
        # the PE clock ramping until the first real matmul arrives.
        # Results are never read.
        wrhs = const.tile([32, 8 * E], BF16)
        nc.vector.memset(wrhs[:], 0.0)
        wpo = pso.tile([128, 2 * 8 * E], F32, tag="po")
        wlhs = offt[:].bitcast(BF16)
        for _ in range(9):
            nc.tensor.matmul(wpo[0:32, 0:512], lhsT=wlhs[:, 0:32],
                             rhs=wrhs[:], start=True, stop=True)

        # evict pattern per group: (ACT, DVE) halves in parallel; two
        # groups go (ACT, ACT) to offload the stream-transpose-loaded DVE
        act_both = {5, 11}

        for i in range(ROWS):
            gsw = gpool.tile([32, 512], BF16)
            for j in range(4):
                # per-j transpose lets each group start right after its
                # own gather; on the in-order DVE queue these interleave
                # with the earlier rows' eviction casts without stalling
                nc.gpsimd.indirect_dma_start(
                    out=gsw[:, 128 * j:128 * (j + 1)], out_offset=None,
                    in_=m2_d[:],
                    in_offset=bass.IndirectOffsetOnAxis(
                        ap=offt[:, 4 * i + j:4 * i + j + 1], axis=0))
                nc.vector.transpose(
                    out=gtrow[0:32, 512 * i + 128 * j:512 * i + 128 * (j + 1)],
                    in_=gsw[:, 128 * j:128 * (j + 1)])
            for j in range(4):
                w = 512 * i + 128 * j
                k = 4 * i + j
                orow = opool.tile([128, 4 * 8 * E], BF16)
                for half in range(2):
                    po = pso.tile([128, 2 * 8 * E], F32, tag="po")
                    for sp in range(2):
                        s = 2 * half + sp
                        nc.tensor.matmul(po[:, 512 * sp:512 * (sp + 1)],
                                         lhsT=gtrow[:, w:w + 128],
                                         rhs=rhs_tiles[s][:],
                                         start=True, stop=True)
                    dst = orow[:, 1024 * half:1024 * (half + 1)]
                    if half == 0 or k in act_both:
                        nc.scalar.copy(out=dst, in_=po[:])
                    else:
                        nc.vector.tensor_copy(out=dst, in_=po[:])
                nc.sync.dma_start(out_d[i, j], orow[:])
    nc.compile()
    return nc


def kernel(traj_loc, mat2, vec, traj_len, l_max, emb_sl_w, emb_su_w,
           emb_tl_w, emb_tu_w):
    import ml_dtypes
    from concourse import bass_utils

    BF = ml_dtypes.bfloat16
    traj_loc = np.asarray(traj_loc).astype(np.int64)
    mat2 = np.ascontiguousarray(np.asarray(mat2, dtype=np.float32))
    traj_len = np.asarray(traj_len).astype(np.int64)
    esl = np.asarray(emb_sl_w, dtype=np.float32)
    esu = np.asarray(emb_su_w, dtype=np.float32)
    etl = np.asarray(emb_tl_w, dtype=np.float32)
    etu = np.asarray(emb_tu_w, dtype=np.float32)

    # host prep: constants
    A = esl + etl                                            # [2, E]
    B = (esu - esl) / np.float32(SU) + (etu - etl) / np.float32(TU)
    mask = (np.arange(M)[None, :] < traj_len[:, None])       # [N, M]
    idx_full = np.where(mask, traj_loc - 1, NLOC).astype(np.int32)

    b1 = B[1].astype(BF)
    dA = (A[1] - A[0]).astype(BF)
    a0 = A[0].astype(BF)

    mat2x = np.concatenate([mat2, np.zeros((1, L), np.float32)], axis=0)
    m2 = np.ascontiguousarray(mat2x.astype(BF))

    # gather offsets: gather (i, j) row-gathers mat2x[idx[i, 32j+p]] into
    # partition p of gsw[:, 128j:128j+128]
    offs = np.empty((NCORES, 32, 4 * ROWS), np.int32)
    for c in range(NCORES):
        for i in range(ROWS):
            idx = idx_full[ROWS * c + i]                     # [128]
            for j in range(4):
                offs[c, :, 4 * i + j] = idx[32 * j:32 * (j + 1)]

    # rhs[s] is [34, 512]: row 8s+lp has B1 in e-block lp (selects the
    # lp-th l within each partition's own l-group); rows 32/33 pair with
    # lhsT rows [m, 1]: out += m*dA + A0, tiled across all 8 e-blocks.
    rhs = np.zeros((4, 128, 8 * E), BF)
    for s in range(4):
        for lp in range(8):
            rhs[s, 8 * s + lp, E * lp:E * (lp + 1)] = b1
        rhs[s, 32, :] = np.tile(dA, 8)
        rhs[s, 33, :] = np.tile(a0, 8)

    # gtrow rows 32:34: col 512i+128j+32g+b must hold m[pos=32j+b] -> the
    # j-th 32-chunk of mask, tiled 4x (over g), per (i, j); rows 2:96 of
    # the staging buffer zero-pad gtrow rows 34:128 for K=128 matmuls
    mrow_full = np.zeros((NCORES, 96, ROWS * 512), BF)
    for c in range(NCORES):
        mc = mask[ROWS * c:ROWS * (c + 1)].astype(BF)        # [ROWS, 128]
        mrow_full[c, 0, :] = np.tile(mc.reshape(ROWS, 4, 1, 32),
                                     (1, 1, 4, 1)).reshape(-1)
        mrow_full[c, 1, :] = 1.0

    if "nc" not in _CACHE:
        _CACHE["nc"] = _build()
    nc = _CACHE["nc"]

    in_maps = []
    for c in range(NCORES):
        in_maps.append({
            "m2": m2,
            "offs": np.ascontiguousarray(offs[c]),
            "mrow": np.ascontiguousarray(mrow_full[c]),
            "rhs": rhs,
        })

    trace = os.environ.get("KERNEL_TRACE", "0") == "1" and _install_profhook()
    res = bass_utils.run_bass_kernel_spmd(
        nc, in_maps, core_ids=list(range(NCORES)), trace=bool(trace))
    if trace:
        _CACHE["exec_time_ns"] = res.exec_time_ns
        _CACHE["trace_path"] = (res.instructions_and_trace or (None, None))[1]
        _CACHE["tmpdir"] = res.profile_json

    # undo the device layout: [i, j, g, b, s, lp, e] -> [i, (j b), (g s lp), e]
    out = np.concatenate(
        [res.results[c]["out"].reshape(ROWS, 4, 4, 32, 4, 8, E)
         .transpose(0, 1, 3, 2, 4, 5, 6).reshape(ROWS, M, L, E)
         for c in range(NCORES)],
        axis=0).astype(np.float32)
    return out


# revision 30
# speedup vs baseline: 1.4708x; 1.0513x over previous
"""Trainium2 Bass kernel for nn_Embed_38766374814290 (embedding_lookup).

Math: out[i,j,l,e] = A[m][e] + delta_s[i,j,l] * B[m][e]
  where m = (j < traj_len[i]), delta_s = where(m, mat2[traj_loc-1], 0),
  A[m] = emb_sl_w[m] + emb_tl_w[m],
  B[m] = (emb_su_w[m]-emb_sl_w[m])/SU + (emb_tu_w[m]-emb_tl_w[m])/TU.

Sharding: pure data parallel over batch N = 32 -> 4 rows per core x 8 cores.

The rel-err gate is 2e-2; bf16 output rounding is ~2^-9, so the device
computes and stores the output in bf16 (halving HBM write traffic vs
fp32 -> ~23us DMA roofline per core) and the host upcasts to fp32.

Per-core pipeline, per batch row i, per 32-position block j:
  1. One indirect row-gather pulls mat2x rows for the 32 positions of
     block j into gsw[32, 128j:+128] (invalid positions redirect to an
     appended all-zero row 4096). No DMAGatherAnt: the gpsimd SWDGE
     descriptor path needs no ucode library, so gathers start ~9.5us.
  2. A DVE stream-transpose (in-place 32x32 blocks) turns the j-window
     into lhsT layout: gt[a, w+32g+b] = ds[pos=32j+b, l=32g+a]. Rows
     32:34 of gtrow ([m, 1] per column) come from one host-prepared
     DMA; they add A[m] inside the matmul.
  3. Matmul (i,j,s): lhsT = the contiguous window [34, 128]; out
     partition f = 32g+b carries (pos=32j+b, l-group g). Two s-matmuls
     (K=34, F=512) per 2-bank PSUM tile, two tiles per group, so the
     Activation and Vector engines evict the halves in parallel
     (~1.15us latency) and 4-tile WAR depth keeps the PE fed.
  4. Output DMA per (i, j) writes the permuted [128, 2048] tile as-is
     (128 x 4KiB contiguous descriptors over all 16 DMA queues); the
     host gather step undoes the (g,b) permutation while upcasting.
"""
import os
import numpy as np
from contextlib import ExitStack

SU, TU = 10000.0, 86400.0
N, M, L, E = 32, 128, 128, 64
NLOC = 4096
NCORES = 8
ROWS = N // NCORES  # 4 batch rows per core

_CACHE = {}


def _install_profhook():
    """Optional: shim the missing antenv.axon_hooks so trace=True works."""
    import sys
    import types
    if "antenv.axon_hooks" in sys.modules:
        return True
    try:
        from trn_agent_boot.trn_boot import _ntff_profile_via_ctypes
    except Exception:
        return False
    hook = [None]
    mod = types.ModuleType("antenv.axon_hooks")
    mod.set_axon_ntff_profile_hook = lambda h: hook.__setitem__(0, h)
    mod.get_axon_ntff_profile_hook = lambda: hook[0]
    sys.modules["antenv.axon_hooks"] = mod
    try:
        mod.set_axon_ntff_profile_hook(
            _ntff_profile_via_ctypes("/opt/axon/libaxon_pjrt.so"))
    except Exception:
        return False
    return True


def _build():
    import concourse.bass as bass
    import concourse.tile as tile
    from concourse import bacc, mybir

    F32 = mybir.dt.float32
    BF16 = mybir.dt.bfloat16
    I32 = mybir.dt.int32

    nc = bacc.Bacc("TRN2", target_bir_lowering=False, debug=False,
                   enable_asserts=True, num_devices=NCORES)
    m2_d = nc.dram_tensor("m2", [NLOC + 1, L], BF16,
                          kind="ExternalInput").ap()
    offs_d = nc.dram_tensor("offs", [64, 2 * ROWS], I32,
                            kind="ExternalInput").ap()
    mrow_d = nc.dram_tensor("mrow", [96, ROWS * 256], BF16,
                            kind="ExternalInput").ap()
    rhs_d = nc.dram_tensor("rhs", [8, 128, 8 * E], BF16,
                           kind="ExternalInput").ap()
    # device-side layout keeps the matmul partition permutation:
    # out[i, j, 32g+b, 512s+64lp+e] = result(pos=32j+b, l=32g+8s+lp, e)
    out_d = nc.dram_tensor("out", [ROWS, 4, M, 4 * 8 * E], BF16,
                           kind="ExternalOutput").ap()

    with tile.TileContext(nc) as tc, ExitStack() as ctx:
        const = ctx.enter_context(tc.tile_pool(name="const", bufs=1))
        gpool = ctx.enter_context(tc.tile_pool(name="gath", bufs=2))
        opool = ctx.enter_context(tc.tile_pool(name="orow", bufs=6))
        pso = ctx.enter_context(tc.tile_pool(name="pso", bufs=4, space="PSUM"))

        offt = const.tile([64, 2 * ROWS], I32)
        nc.sync.dma_start(offt[:], offs_d[:])
        # gtrow: one 128-col window per gather PAIR (i, c); partitions
        # 0:32 = even-j G^T, 32:64 = odd-j G^T, 64:68 = [m_e, 1, m_o, 1],
        # 68:128 zero. K=128 matmuls read the full window; two rhs
        # variants select the even or odd band (K<=64 tiles would run at
        # half column throughput, so always use the full array).
        gtrow = const.tile([128, ROWS * 256], BF16)
        nc.scalar.dma_start(gtrow[64:128, :], mrow_d[32:96, :])
        rhs_tiles = []
        for v in range(8):
            rt = const.tile([128, 8 * E], BF16, tag=f"rhs{v}")
            nc.scalar.dma_start(rt[:], rhs_d[v])
            rhs_tiles.append(rt)

        # HAM warmup: a few matmuls gated on the offs DMA (~9.5us) keep
        # the PE clock ramping until the first real matmul arrives.
        # Results are never read.
        wrhs = const.tile([64, 8 * E], BF16)
        nc.vector.memset(wrhs[:], 0.0)
        wpo = pso.tile([128, 2 * 8 * E], F32, tag="po")
        wlhs = offt[:].bitcast(BF16)
        for _ in range(9):
            nc.tensor.matmul(wpo[0:16, 0:512], lhsT=wlhs[:, 0:16],
                             rhs=wrhs[:], start=True, stop=True)

        # evict pattern per group: (ACT, DVE) halves in parallel; two
        # groups go (ACT, ACT) to offload the stream-transpose-loaded DVE
        act_both = {5, 11}

        for i in range(ROWS):
            gsw = gpool.tile([64, 256], BF16)
            for c in range(2):
                # each pair-gather pulls 64 positions (two j-blocks on
                # partitions 0:32 / 32:64); the transpose interleaves
                # with eviction casts on the in-order DVE queue
                nc.gpsimd.indirect_dma_start(
                    out=gsw[:, 128 * c:128 * (c + 1)], out_offset=None,
                    in_=m2_d[:],
                    in_offset=bass.IndirectOffsetOnAxis(
                        ap=offt[:, 2 * i + c:2 * i + c + 1], axis=0))
                nc.vector.transpose(
                    out=gtrow[0:64, 256 * i + 128 * c:256 * i + 128 * (c + 1)],
                    in_=gsw[:, 128 * c:128 * (c + 1)])
            for j in range(4):
                c, pb = j >> 1, j & 1
                w = 256 * i + 128 * c
                k = 4 * i + j
                orow = opool.tile([128, 4 * 8 * E], BF16)
                for half in range(2):
                    po = pso.tile([128, 2 * 8 * E], F32, tag="po")
                    for sp in range(2):
                        s = 2 * half + sp
                        nc.tensor.matmul(po[:, 512 * sp:512 * (sp + 1)],
                                         lhsT=gtrow[:, w:w + 128],
                                         rhs=rhs_tiles[4 * pb + s][:],
                                         start=True, stop=True)
                    dst = orow[:, 1024 * half:1024 * (half + 1)]
                    if half == 0 or k in act_both:
                        nc.scalar.copy(out=dst, in_=po[:])
                    else:
                        nc.vector.tensor_copy(out=dst, in_=po[:])
                nc.sync.dma_start(out_d[i, j], orow[:])
    nc.compile()
    return nc


def kernel(traj_loc, mat2, vec, traj_len, l_max, emb_sl_w, emb_su_w,
           emb_tl_w, emb_tu_w):
    import ml_dtypes
    from concourse import bass_utils

    BF = ml_dtypes.bfloat16
    traj_loc = np.asarray(traj_loc).astype(np.int64)
    mat2 = np.ascontiguousarray(np.asarray(mat2, dtype=np.float32))
    traj_len = np.asarray(traj_len).astype(np.int64)
    esl = np.asarray(emb_sl_w, dtype=np.float32)
    esu = np.asarray(emb_su_w, dtype=np.float32)
    etl = np.asarray(emb_tl_w, dtype=np.float32)
    etu = np.asarray(emb_tu_w, dtype=np.float32)

    # host prep: constants
    A = esl + etl                                            # [2, E]
    B = (esu - esl) / np.float32(SU) + (etu - etl) / np.float32(TU)
    mask = (np.arange(M)[None, :] < traj_len[:, None])       # [N, M]
    idx_full = np.where(mask, traj_loc - 1, NLOC).astype(np.int32)

    b1 = B[1].astype(BF)
    dA = (A[1] - A[0]).astype(BF)
    a0 = A[0].astype(BF)

    mat2x = np.concatenate([mat2, np.zeros((1, L), np.float32)], axis=0)
    m2 = np.ascontiguousarray(mat2x.astype(BF))

    # gather offsets: pair-gather (i, c) row-gathers mat2x[idx[i, 64c+p]]
    # into partition p (0:64) of gsw[:, 128c:128c+128]
    offs = np.empty((NCORES, 64, 2 * ROWS), np.int32)
    for cc in range(NCORES):
        for i in range(ROWS):
            idx = idx_full[ROWS * cc + i]                    # [128]
            for c in range(2):
                offs[cc, :, 2 * i + c] = idx[64 * c:64 * (c + 1)]

    # rhs[4*pb+s]: variant pb selects the even (rows 0:32) or odd (rows
    # 32:64) G^T band: row 32*pb+8s+lp has B1 in e-block lp; the [m, 1]
    # selectors at rows 64+2*pb : 66+2*pb add m*dA + A0 per e-block.
    rhs = np.zeros((8, 128, 8 * E), BF)
    for pb in range(2):
        for s in range(4):
            v = 4 * pb + s
            for lp in range(8):
                rhs[v, 32 * pb + 8 * s + lp, E * lp:E * (lp + 1)] = b1
            rhs[v, 64 + 2 * pb, :] = np.tile(dA, 8)
            rhs[v, 65 + 2 * pb, :] = np.tile(a0, 8)

    # gtrow rows 64:68 per (i, c) window: [m_even, 1, m_odd, 1] where
    # m_j col 32g+b = mask[pos=32j+b] tiled over g; the rest of the
    # staging buffer zero-pads gtrow rows 68:128 for K=128 matmuls.
    # mrow staging covers gtrow rows 32:128 -> buffer rows 32:36 hold it.
    mrow_full = np.zeros((NCORES, 96, ROWS * 256), BF)
    for cc in range(NCORES):
        mc = mask[ROWS * cc:ROWS * (cc + 1)].astype(BF)      # [ROWS, 128]
        mj = np.tile(mc.reshape(ROWS, 4, 1, 32), (1, 1, 4, 1))  # [R,4j,4g,32]
        mjf = mj.reshape(ROWS, 4, 128)                       # [R, j, 128]
        for i in range(ROWS):
            for c in range(2):
                w = 256 * i + 128 * c
                mrow_full[cc, 32, w:w + 128] = mjf[i, 2 * c]
                mrow_full[cc, 33, w:w + 128] = 1.0
                mrow_full[cc, 34, w:w + 128] = mjf[i, 2 * c + 1]
                mrow_full[cc, 35, w:w + 128] = 1.0

    if "nc" not in _CACHE:
        _CACHE["nc"] = _build()
    nc = _CACHE["nc"]

    in_maps = []
    for c in range(NCORES):
        in_maps.append({
            "m2": m2,
            "offs": np.ascontiguousarray(offs[c]),
            "mrow": np.ascontiguousarray(mrow_full[c]),
            "rhs": rhs,
        })

    trace = os.environ.get("KERNEL_TRACE", "0") == "1" and _install_profhook()
    res = bass_utils.run_bass_kernel_spmd(
        nc, in_maps, core_ids=list(range(NCORES)), trace=bool(trace))
    if trace:
        _CACHE["exec_time_ns"] = res.exec_time_ns
        _CACHE["trace_path"] = (res.instructions_and_trace or (None, None))[1]
        _CACHE["tmpdir"] = res.profile_json

    # undo the device layout: [i, j, g, b, s, lp, e] -> [i, (j b), (g s lp), e]
    out = np.concatenate(
        [res.results[c]["out"].reshape(ROWS, 4, 4, 32, 4, 8, E)
         .transpose(0, 1, 3, 2, 4, 5, 6).reshape(ROWS, M, L, E)
         for c in range(NCORES)],
        axis=0).astype(np.float32)
    return out


# revision 32
# speedup vs baseline: 1.5769x; 1.0721x over previous
"""Trainium2 Bass kernel for nn_Embed_38766374814290 (embedding_lookup).

Math: out[i,j,l,e] = A[m][e] + delta_s[i,j,l] * B[m][e]
  where m = (j < traj_len[i]), delta_s = where(m, mat2[traj_loc-1], 0),
  A[m] = emb_sl_w[m] + emb_tl_w[m],
  B[m] = (emb_su_w[m]-emb_sl_w[m])/SU + (emb_tu_w[m]-emb_tl_w[m])/TU.

Sharding: pure data parallel over batch N = 32 -> 4 rows per core x 8 cores.

The rel-err gate is 2e-2; bf16 output rounding is ~2^-9, so the device
computes and stores the output in bf16 (halving HBM write traffic vs
fp32 -> ~23us DMA roofline per core) and the host upcasts to fp32.

Per-core pipeline, per batch row i, per pair c of 32-position blocks:
  1. One indirect pair-gather pulls mat2x rows for 64 positions into
     gsw[0:64, 128c:+128] (invalid positions redirect to an appended
     all-zero row 4096). The SWDGE descriptor path needs no gpsimd
     ucode library, so gathers start ~9.5us (DMAGatherAnt stalls ~12us
     on a lazy library load).
  2. A DVE stream-transpose (in-place 32x32 blocks) turns the window
     into lhsT layout: even j at partitions 0:32, odd j at 32:64. One
     host-prepared DMA fills rows 64:68 = [m_even, 1, m_odd, 1] and
     zero-pads rows 68:128.
  3. Matmul (i,j,s): lhsT = the full [128, 128] window (K=128: K<=64
     tiles run at half PE column throughput); one of 8 rhs variants
     selects the even/odd band and adds A[m] via the m/1 rows. Out
     partition f = 32g+b carries (pos=32j+b, l-group g). Two s-matmuls
     (F=512) per 2-bank PSUM tile, two tiles per group, so the
     Activation and Vector engines evict the halves in parallel and
     4-tile WAR depth keeps the PE fed.
  4. Output DMA per (i, j) writes the permuted [128, 2048] tile as-is
     (128 x 4KiB contiguous descriptors over all 16 DMA queues); the
     host gather step undoes the (g,b) permutation while upcasting.
"""
import os
import numpy as np
from contextlib import ExitStack

SU, TU = 10000.0, 86400.0
N, M, L, E = 32, 128, 128, 64
NLOC = 4096
NCORES = 8
ROWS = N // NCORES  # 4 batch rows per core

_CACHE = {}


def _install_profhook():
    """Optional: shim the missing antenv.axon_hooks so trace=True works."""
    import sys
    import types
    if "antenv.axon_hooks" in sys.modules:
        return True
    try:
        from trn_agent_boot.trn_boot import _ntff_profile_via_ctypes
    except Exception:
        return False
    hook = [None]
    mod = types.ModuleType("antenv.axon_hooks")
    mod.set_axon_ntff_profile_hook = lambda h: hook.__setitem__(0, h)
    mod.get_axon_ntff_profile_hook = lambda: hook[0]
    sys.modules["antenv.axon_hooks"] = mod
    try:
        mod.set_axon_ntff_profile_hook(
            _ntff_profile_via_ctypes("/opt/axon/libaxon_pjrt.so"))
    except Exception:
        return False
    return True


def _build():
    import concourse.bass as bass
    import concourse.tile as tile
    from concourse import bacc, mybir

    F32 = mybir.dt.float32
    BF16 = mybir.dt.bfloat16
    I32 = mybir.dt.int32

    nc = bacc.Bacc("TRN2", target_bir_lowering=False, debug=False,
                   enable_asserts=False, num_devices=NCORES)
    m2_d = nc.dram_tensor("m2", [NLOC + 1, L], BF16,
                          kind="ExternalInput").ap()
    offs_d = nc.dram_tensor("offs", [64, 2 * ROWS], I32,
                            kind="ExternalInput").ap()
    mrow_d = nc.dram_tensor("mrow", [96, ROWS * 256], BF16,
                            kind="ExternalInput").ap()
    rhs_d = nc.dram_tensor("rhs", [8, 128, 8 * E], BF16,
                           kind="ExternalInput").ap()
    # device-side layout keeps the matmul partition permutation:
    # out[i, j, 32g+b, 512s+64lp+e] = result(pos=32j+b, l=32g+8s+lp, e)
    out_d = nc.dram_tensor("out", [ROWS, 4, M, 4 * 8 * E], BF16,
                           kind="ExternalOutput").ap()

    with tile.TileContext(nc) as tc, ExitStack() as ctx:
        const = ctx.enter_context(tc.tile_pool(name="const", bufs=1))
        gpool = ctx.enter_context(tc.tile_pool(name="gath", bufs=2))
        opool = ctx.enter_context(tc.tile_pool(name="orow", bufs=6))
        pso = ctx.enter_context(tc.tile_pool(name="pso", bufs=4, space="PSUM"))

        offt = const.tile([64, 2 * ROWS], I32)
        nc.scalar.dma_start(offt[:], offs_d[:])
        # gtrow: one 128-col window per gather PAIR (i, c); partitions
        # 0:32 = even-j G^T, 32:64 = odd-j G^T, 64:68 = [m_e, 1, m_o, 1],
        # 68:128 zero. K=128 matmuls read the full window; two rhs
        # variants select the even or odd band (K<=64 tiles would run at
        # half column throughput, so always use the full array).
        gtrow = const.tile([128, ROWS * 256], BF16)
        nc.scalar.dma_start(gtrow[64:128, :], mrow_d[32:96, :])
        rhs_tiles = []
        for v in range(8):
            rt = const.tile([128, 8 * E], BF16, tag=f"rhs{v}")
            nc.scalar.dma_start(rt[:], rhs_d[v])
            rhs_tiles.append(rt)

        # HAM warmup: a few matmuls gated on the offs DMA (~9.5us) keep
        # the PE clock ramping until the first real matmul arrives.
        # Results are never read.
        wrhs = const.tile([64, 8 * E], BF16)
        nc.vector.memset(wrhs[:], 0.0)
        wpo = pso.tile([128, 2 * 8 * E], F32, tag="po")
        wlhs = offt[:].bitcast(BF16)
        for _ in range(9):
            nc.tensor.matmul(wpo[0:16, 0:512], lhsT=wlhs[:, 0:16],
                             rhs=wrhs[:], start=True, stop=True)

        # evict pattern per group: (ACT, DVE) halves in parallel; two
        # groups go (ACT, ACT) to offload the stream-transpose-loaded DVE
        act_both = {6}

        for i in range(ROWS):
            gsw = gpool.tile([64, 256], BF16)
            for c in range(2):
                # each pair-gather pulls 64 positions (two j-blocks on
                # partitions 0:32 / 32:64); the transpose interleaves
                # with eviction casts on the in-order DVE queue
                nc.gpsimd.indirect_dma_start(
                    out=gsw[:, 128 * c:128 * (c + 1)], out_offset=None,
                    in_=m2_d[:],
                    in_offset=bass.IndirectOffsetOnAxis(
                        ap=offt[:, 2 * i + c:2 * i + c + 1], axis=0))
                nc.vector.transpose(
                    out=gtrow[0:64, 256 * i + 128 * c:256 * i + 128 * (c + 1)],
                    in_=gsw[:, 128 * c:128 * (c + 1)])
            for j in range(4):
                c, pb = j >> 1, j & 1
                w = 256 * i + 128 * c
                k = 4 * i + j
                orow = opool.tile([128, 4 * 8 * E], BF16)
                for half in range(2):
                    po = pso.tile([128, 2 * 8 * E], F32, tag="po")
                    for sp in range(2):
                        s = 2 * half + sp
                        nc.tensor.matmul(po[:, 512 * sp:512 * (sp + 1)],
                                         lhsT=gtrow[:, w:w + 128],
                                         rhs=rhs_tiles[4 * pb + s][:],
                                         start=True, stop=True)
                    dst = orow[:, 1024 * half:1024 * (half + 1)]
                    if half == 0 or k in act_both:
                        nc.scalar.copy(out=dst, in_=po[:])
                    else:
                        nc.vector.tensor_copy(out=dst, in_=po[:])
                nc.sync.dma_start(out_d[i, j], orow[:])
    nc.compile()
    return nc


def kernel(traj_loc, mat2, vec, traj_len, l_max, emb_sl_w, emb_su_w,
           emb_tl_w, emb_tu_w):
    import ml_dtypes
    from concourse import bass_utils

    BF = ml_dtypes.bfloat16
    traj_loc = np.asarray(traj_loc).astype(np.int64)
    mat2 = np.ascontiguousarray(np.asarray(mat2, dtype=np.float32))
    traj_len = np.asarray(traj_len).astype(np.int64)
    esl = np.asarray(emb_sl_w, dtype=np.float32)
    esu = np.asarray(emb_su_w, dtype=np.float32)
    etl = np.asarray(emb_tl_w, dtype=np.float32)
    etu = np.asarray(emb_tu_w, dtype=np.float32)

    # host prep: constants
    A = esl + etl                                            # [2, E]
    B = (esu - esl) / np.float32(SU) + (etu - etl) / np.float32(TU)
    mask = (np.arange(M)[None, :] < traj_len[:, None])       # [N, M]
    idx_full = np.where(mask, traj_loc - 1, NLOC).astype(np.int32)

    b1 = B[1].astype(BF)
    dA = (A[1] - A[0]).astype(BF)
    a0 = A[0].astype(BF)

    mat2x = np.concatenate([mat2, np.zeros((1, L), np.float32)], axis=0)
    m2 = np.ascontiguousarray(mat2x.astype(BF))

    # gather offsets: pair-gather (i, c) row-gathers mat2x[idx[i, 64c+p]]
    # into partition p (0:64) of gsw[:, 128c:128c+128]
    offs = np.empty((NCORES, 64, 2 * ROWS), np.int32)
    for cc in range(NCORES):
        for i in range(ROWS):
            idx = idx_full[ROWS * cc + i]                    # [128]
            for c in range(2):
                offs[cc, :, 2 * i + c] = idx[64 * c:64 * (c + 1)]

    # rhs[4*pb+s]: variant pb selects the even (rows 0:32) or odd (rows
    # 32:64) G^T band: row 32*pb+8s+lp has B1 in e-block lp; the [m, 1]
    # selectors at rows 64+2*pb : 66+2*pb add m*dA + A0 per e-block.
    rhs = np.zeros((8, 128, 8 * E), BF)
    for pb in range(2):
        for s in range(4):
            v = 4 * pb + s
            for lp in range(8):
                rhs[v, 32 * pb + 8 * s + lp, E * lp:E * (lp + 1)] = b1
            rhs[v, 64 + 2 * pb, :] = np.tile(dA, 8)
            rhs[v, 65 + 2 * pb, :] = np.tile(a0, 8)

    # gtrow rows 64:68 per (i, c) window: [m_even, 1, m_odd, 1] where
    # m_j col 32g+b = mask[pos=32j+b] tiled over g; the rest of the
    # staging buffer zero-pads gtrow rows 68:128 for K=128 matmuls.
    # mrow staging covers gtrow rows 32:128 -> buffer rows 32:36 hold it.
    mrow_full = np.zeros((NCORES, 96, ROWS * 256), BF)
    for cc in range(NCORES):
        mc = mask[ROWS * cc:ROWS * (cc + 1)].astype(BF)      # [ROWS, 128]
        mj = np.tile(mc.reshape(ROWS, 4, 1, 32), (1, 1, 4, 1))  # [R,4j,4g,32]
        mjf = mj.reshape(ROWS, 4, 128)                       # [R, j, 128]
        for i in range(ROWS):
            for c in range(2):
                w = 256 * i + 128 * c
                mrow_full[cc, 32, w:w + 128] = mjf[i, 2 * c]
                mrow_full[cc, 33, w:w + 128] = 1.0
                mrow_full[cc, 34, w:w + 128] = mjf[i, 2 * c + 1]
                mrow_full[cc, 35, w:w + 128] = 1.0

    if "nc" not in _CACHE:
        _CACHE["nc"] = _build()
    nc = _CACHE["nc"]

    in_maps = []
    for c in range(NCORES):
        in_maps.append({
            "m2": m2,
            "offs": np.ascontiguousarray(offs[c]),
            "mrow": np.ascontiguousarray(mrow_full[c]),
            "rhs": rhs,
        })

    trace = os.environ.get("KERNEL_TRACE", "0") == "1" and _install_profhook()
    res = bass_utils.run_bass_kernel_spmd(
        nc, in_maps, core_ids=list(range(NCORES)), trace=bool(trace))
    if trace:
        _CACHE["exec_time_ns"] = res.exec_time_ns
        _CACHE["trace_path"] = (res.instructions_and_trace or (None, None))[1]
        _CACHE["tmpdir"] = res.profile_json

    # undo the device layout: [i, j, g, b, s, lp, e] -> [i, (j b), (g s lp), e]
    out = np.concatenate(
        [res.results[c]["out"].reshape(ROWS, 4, 4, 32, 4, 8, E)
         .transpose(0, 1, 3, 2, 4, 5, 6).reshape(ROWS, M, L, E)
         for c in range(NCORES)],
        axis=0).astype(np.float32)
    return out
